# revision 1
# baseline (speedup 1.0000x reference)
"""Bass/Tile kernel for nn_MultiHeadAttention_84104049590613 on 8 trn2 cores.

Sharding: core c in 0..7 handles batch b = c//4 and query rows
[qb*512, (qb+1)*512) with qb = c%4, for ALL 8 heads.  mask/dict_mask shard
along (b, q); key/value duplicate along the 4 q-block cores of a batch.

Score layout on chip is transposed: [k (partition), q (free)] so that the
attention*V matmul contracts k on the partition dim directly and the softmax
denominator comes free as a ones-column appended to V.  Softmax uses no
max-subtraction (scores are O(5); exp never overflows) and the 0/1 mask is
applied multiplicatively after exp.
"""

import numpy as np

import concourse.bass as bass
import concourse.mybir as mybir
import concourse.tile as tile
from concourse.bass_utils import run_bass_kernel_spmd

dt = mybir.dt
Alu = mybir.AluOpType
Act = mybir.ActivationFunctionType

B, S, E, H, DH = 2, 2048, 512, 8, 64
SQ = 512            # query rows per core
NCORE = 8
NKT = S // 128      # 16 k tiles
NQT = SQ // 128     # 4 q tiles
NEC = E // 128      # 4 e chunks
NST = S // 128      # 16 s tiles


def split_multi_waits(nc):
    """walrus in this container accepts a single sync-wait command per
    instruction; Tile's tail drain can carry several.  Peel extras onto
    preceding NoOps."""
    def fix_bb(bb):
        insts = list(bb.instructions)
        if not any(i.sync_info and i.sync_info.on_wait and len(i.sync_info.on_wait) > 1
                   for i in insts):
            return
        new = []
        for inst in insts:
            si = inst.sync_info
            if si and si.on_wait and len(si.on_wait) > 1:
                waits = list(si.on_wait)
                for w in waits[:-1]:
                    new.append(mybir.InstNoOp(
                        name=nc.get_next_instruction_name(),
                        engine=inst.engine,
                        bass_nofuse=True,
                        sync_info=mybir.SyncInfo(on_wait=[w], on_update=[]),
                    ))
                inst.sync_info = mybir.SyncInfo(
                    on_wait=[waits[-1]], on_update=list(si.on_update or []))
            new.append(inst)
        bb.instructions = new

    for f in nc.m.functions:
        for bb in f.blocks:
            fix_bb(bb)


def build(waitfix=True, upto='all'):
    nc = bass.Bass()

    query_s = nc.dram_tensor("query_s", [SQ, E], dt.float32, kind="ExternalInput")
    key_b = nc.dram_tensor("key_b", [S, E], dt.float32, kind="ExternalInput")
    value_b = nc.dram_tensor("value_b", [S, E], dt.float32, kind="ExternalInput")
    mask_s = nc.dram_tensor("mask_s", [SQ, S], dt.int32, kind="ExternalInput")
    d0_s = nc.dram_tensor("d0_s", [SQ, S], dt.float32, kind="ExternalInput")
    d1_s = nc.dram_tensor("d1_s", [SQ, S], dt.float32, kind="ExternalInput")
    wq_d = nc.dram_tensor("wq_d", [E, E], dt.float32, kind="ExternalInput")
    wk_d = nc.dram_tensor("wk_d", [E, E], dt.float32, kind="ExternalInput")
    wv_d = nc.dram_tensor("wv_d", [E, E], dt.float32, kind="ExternalInput")
    wo_d = nc.dram_tensor("wo_d", [E, E], dt.float32r, kind="ExternalInput")
    # consts columns (broadcast over partitions): h -> ratio r_h; 8+h -> scale_h
    consts_d = nc.dram_tensor("consts_d", [128, 16], dt.float32, kind="ExternalInput")
    bqs_d = nc.dram_tensor("bqs_d", [128, NEC], dt.float32, kind="ExternalInput")
    bks_d = nc.dram_tensor("bks_d", [128, NEC], dt.float32, kind="ExternalInput")
    crow_d = nc.dram_tensor("crow_d", [1, E], dt.float32r, kind="ExternalInput")
    sel_d = nc.dram_tensor("sel_d", [8, H * 64], dt.float32r, kind="ExternalInput")
    eye_d = nc.dram_tensor("eye_d", [1, H * 8], dt.float32r, kind="ExternalInput")
    ones_d = nc.dram_tensor("ones_d", [1, 128], dt.float32r, kind="ExternalInput")
    out_d = nc.dram_tensor("out_d", [SQ, E], dt.float32, kind="ExternalOutput")

    with tile.TileContext(nc) as tc, tc.tile_pool(name="persist", bufs=1) as pp:
        # ---------------- persistent tiles ----------------
        d0T = pp.tile([128, NKT * SQ], dt.bfloat16)      # [k%128, kt*512+q]
        d1T = pp.tile([128, NKT * SQ], dt.bfloat16)
        maskT = pp.tile([128, NKT * SQ], dt.float16)
        kTp = [pp.tile([128, S], dt.float16, name=f"kTp{i}", tag=f"kTp{i}") for i in range(4)]
        qTp = [pp.tile([128, SQ], dt.float16, name=f"qTp{i}", tag=f"qTp{i}") for i in range(4)]
        vaug = [pp.tile([128, H * 65], dt.float16, name=f"va{i}", tag=f"va{i}") for i in range(NST)]
        oT = [pp.tile([64, SQ], dt.float32r, name=f"oT{i}", tag=f"oT{i}") for i in range(H)]
        consts = pp.tile([128, 16], dt.float32)
        bqs = pp.tile([128, NEC], dt.float32)
        bks = pp.tile([128, NEC], dt.float32)
        eye = pp.tile([65, H * 8], dt.float32r)          # row 64: unit selectors
        sel = pp.tile([8, H * 64], dt.float32r)          # head -> 64-row bcast
        rcp = pp.tile([8, SQ], dt.float32r)

        nc.scalar.dma_start(consts[:], consts_d[:, :])
        nc.scalar.dma_start(bqs[:], bqs_d[:, :])
        nc.scalar.dma_start(bks[:], bks_d[:, :])
        nc.scalar.dma_start(eye[64:65, :], eye_d[:, :])
        nc.scalar.dma_start(sel[:], sel_d[:, :])
        for st in range(NST):
            nc.gpsimd.memset(
                vaug[st][:].rearrange("p (h x) -> p h x", h=H)[:, :, 64:65], 1.0)

        # ---------------- loads, transposes, projections ----------------
        with tc.tile_pool(name="stage", bufs=1) as stg, \
             tc.tile_pool(name="stg2", bufs=1) as stg2, \
             tc.tile_pool(name="stg3", bufs=2) as stg3, \
             tc.tile_pool(name="proj_ps", bufs=4, space="PSUM") as pjp:
            # query: fp16 natural staging via gpsimd cast DMA, then XBAR
            qf = stg.tile([128, NQT * E], dt.float16, tag="qf")
            nc.gpsimd.dma_start(qf[:].rearrange("p (st e) -> p st e", st=NQT),
                                query_s.rearrange("(st p) e -> p st e", p=128))
            qTin = stg.tile([128, NEC * SQ], dt.float16, tag="qTin")
            for st in range(NQT):
                nc.sync.dma_start(
                    qTin[:].rearrange("p (ec q) -> p ec q", ec=NEC)[:, :, st * 128:(st + 1) * 128],
                    qf[:, st * E:(st + 1) * E], transpose=True)
            # weights fp16 natural
            wqf = stg.tile([128, NEC * E], dt.float16, tag="wqf")
            wkf = stg.tile([128, NEC * E], dt.float16, tag="wkf")
            wvf = stg.tile([128, NEC * E], dt.float16, tag="wvf")
            for wt, wd in ((wqf, wq_d), (wkf, wk_d), (wvf, wv_d)):
                nc.gpsimd.dma_start(wt[:].rearrange("p (ec e) -> p ec e", ec=NEC),
                                    wd.rearrange("(ec p) e -> p ec e", p=128))

            # q^T projection (pre-scaled by 1/sqrt(DH) = 0.125)
            do_proj = upto in ('proj', 'att', 'all')
            for hp in range(4 if do_proj else 0):
                ps = pjp.tile([128, SQ], dt.float32, tag="pproj")
                for ec in range(NEC):
                    nc.tensor.matmul(
                        ps[:], wqf[:, ec * E + hp * 128: ec * E + (hp + 1) * 128],
                        qTin[:, ec * SQ:(ec + 1) * SQ],
                        start=(ec == 0), stop=(ec == NEC - 1))
                nc.scalar.activation(qTp[hp][:], ps[:], Act.Identity,
                                     bias=bqs[:, hp:hp + 1], scale=0.125)

            # key/value in halves: cast-load natural, XBAR, project, release
            for half in range(2):
                kf = stg2.tile([128, 8 * E], dt.float16, tag="kf")
                vf = stg2.tile([128, 8 * E], dt.float16, tag="vf")
                sl = slice(half * 1024, half * 1024 + 1024)
                nc.gpsimd.dma_start(
                    kf[:].rearrange("p (st e) -> p st e", st=8),
                    key_b[sl].rearrange("(st p) e -> p st e", p=128))
                nc.gpsimd.dma_start(
                    vf[:].rearrange("p (st e) -> p st e", st=8),
                    value_b[sl].rearrange("(st p) e -> p st e", p=128))
                kTin = stg2.tile([128, NEC * 1024], dt.float16, tag="kTin")
                vTin = stg2.tile([128, NEC * 1024], dt.float16, tag="vTin")
                for st8 in range(8):
                    nc.sync.dma_start(
                        kTin[:].rearrange("p (ec s) -> p ec s", ec=NEC)[:, :, st8 * 128:(st8 + 1) * 128],
                        kf[:, st8 * E:(st8 + 1) * E], transpose=True)
                    nc.sync.dma_start(
                        vTin[:].rearrange("p (ec s) -> p ec s", ec=NEC)[:, :, st8 * 128:(st8 + 1) * 128],
                        vf[:, st8 * E:(st8 + 1) * E], transpose=True)
                # k^T projection for this half (s columns half*1024 ..)
                for hp in range(4 if do_proj else 0):
                    for sc in range(2):
                        ps = pjp.tile([128, 512], dt.float32, tag="pproj")
                        for ec in range(NEC):
                            nc.tensor.matmul(
                                ps[:],
                                wkf[:, ec * E + hp * 128: ec * E + (hp + 1) * 128],
                                kTin[:, ec * 1024 + sc * 512: ec * 1024 + (sc + 1) * 512],
                                start=(ec == 0), stop=(ec == NEC - 1))
                            pass
                        nc.scalar.activation(
                            kTp[hp][:, half * 1024 + sc * 512: half * 1024 + (sc + 1) * 512],
                            ps[:], Act.Identity, bias=bks[:, hp:hp + 1])
                # v projection for this half
                for st8 in range(8 if do_proj else 0):
                    st = half * 8 + st8
                    ps = pjp.tile([128, E], dt.float32, tag="pproj")
                    for ec in range(NEC):
                        nc.tensor.matmul(
                            ps[:],
                            vTin[:, ec * 1024 + st8 * 128: ec * 1024 + (st8 + 1) * 128],
                            wvf[:, ec * E:(ec + 1) * E],
                            start=(ec == 0), stop=(ec == NEC - 1))
                    nc.scalar.activation(
                        vaug[st][:].rearrange("p (h x) -> p h x", h=H)[:, :, 0:64],
                        ps[:], Act.Identity)

            # mask: int32 natural -> f16 natural (DVE cast) -> XBAR
            for qt in range(NQT):
                mi = stg3.tile([128, S], dt.int32, tag="mi")
                nc.scalar.dma_start(
                    mi[:], mask_s.rearrange("(qt p) k -> qt p k", p=128)[qt])
                mfc = stg3.tile([128, S], dt.float16, tag="mfc")
                nc.vector.tensor_copy(mfc[:], mi[:])
                nc.sync.dma_start(
                    maskT[:].rearrange("p (kt q) -> p kt q", kt=NKT)[:, :, qt * 128:(qt + 1) * 128],
                    mfc[:], transpose=True)
            # dict_mask channels: bf16 natural via cast DMA -> XBAR
            for qt in range(NQT):
                d0c = stg3.tile([128, S], dt.bfloat16, tag="d0c")
                d1c = stg3.tile([128, S], dt.bfloat16, tag="d1c")
                nc.gpsimd.dma_start(
                    d0c[:], d0_s.rearrange("(qt p) k -> qt p k", p=128)[qt])
                nc.gpsimd.dma_start(
                    d1c[:], d1_s.rearrange("(qt p) k -> qt p k", p=128)[qt])
                nc.sync.dma_start(
                    d0T[:].rearrange("p (kt q) -> p kt q", kt=NKT)[:, :, qt * 128:(qt + 1) * 128],
                    d0c[:], transpose=True)
                nc.sync.dma_start(
                    d1T[:].rearrange("p (kt q) -> p kt q", kt=NKT)[:, :, qt * 128:(qt + 1) * 128],
                    d1c[:], transpose=True)

        # ---------------- attention ----------------
        with tc.tile_pool(name="dall_ps", bufs=1, space="PSUM") as dap:
          dall = dap.tile([8, SQ], dt.float32)
          with tc.tile_pool(name="att", bufs=2) as att, \
               tc.tile_pool(name="attp", bufs=3) as attp, \
               tc.tile_pool(name="den", bufs=2) as denp, \
               tc.tile_pool(name="qk_ps", bufs=3, space="PSUM") as qkp, \
               tc.tile_pool(name="av_ps", bufs=2, space="PSUM") as avp:
            for h in range(H if upto in ('att', 'all') else 0):
                hp, hsub = h // 2, h % 2
                qT_h = qTp[hp][hsub * 64:(hsub + 1) * 64, :]
                r_ap = consts[:, h:h + 1]
                s_ap = consts[:, 8 + h:8 + h + 1]
                av = avp.tile([65, SQ], dt.float32, tag="av")
                for hf in range(2):  # half-head granularity for SBUF
                    y = att.tile([128, 8 * SQ], dt.bfloat16, tag="y")
                    nc.vector.scalar_tensor_tensor(
                        y[:], d1T[:, hf * 8 * SQ:(hf + 1) * 8 * SQ], r_ap,
                        d0T[:, hf * 8 * SQ:(hf + 1) * 8 * SQ], Alu.mult, Alu.add)
                    edm = att.tile([128, 8 * SQ], dt.bfloat16, tag="edm")
                    nc.scalar.activation(edm[:], y[:], Act.Exp, scale=s_ap)
                    for g in range(2):  # groups of 4 k-tiles
                        sn = attp.tile([128, 4 * SQ], dt.bfloat16, tag="sn")
                        for i in range(4):
                            kt = hf * 8 + g * 4 + i
                            qk = qkp.tile([128, SQ], dt.float32, tag="qk")
                            nc.tensor.matmul(
                                qk[:], kTp[hp][hsub * 64:(hsub + 1) * 64,
                                               kt * 128:(kt + 1) * 128],
                                qT_h, start=True, stop=True)
                            nc.vector.scalar_tensor_tensor(
                                sn[:, i * SQ:(i + 1) * SQ],
                                edm[:, (g * 4 + i) * SQ:(g * 4 + i + 1) * SQ],
                                1.0, qk[:], Alu.mult, Alu.subtract)
                        pgrp = attp.tile([128, 4 * SQ], dt.float16, tag="pgrp")
                        nc.scalar.activation(pgrp[:], sn[:], Act.Exp, scale=-1.0)
                        for i in range(4):
                            kt = hf * 8 + g * 4 + i
                            pm = attp.tile([128, SQ], dt.float16, tag="pm")
                            nc.gpsimd.tensor_tensor(
                                pm[:], pgrp[:, i * SQ:(i + 1) * SQ],
                                maskT[:, kt * SQ:(kt + 1) * SQ], Alu.mult)
                            nc.tensor.matmul(
                                av[:],
                                vaug[kt][:].rearrange("p (hh x) -> p hh x", hh=H)[:, h, :],
                                pm[:], start=(kt == 0), stop=(kt == NKT - 1))
                # attention rows -> per-head sbuf; denominator -> dall row h
                nc.vector.tensor_copy(oT[h][:], av[0:64, :])
                den = denp.tile([65, SQ], dt.float32r, tag="den")
                nc.vector.tensor_copy(den[64:65, :], av[64:65, :])
                nc.tensor.matmul(dall[:], eye[64:65, h * 8:(h + 1) * 8],
                                 den[64:65, :], start=(h == 0), stop=(h == H - 1))

          # ---------------- normalize + output projection ----------------
          with tc.tile_pool(name="fin", bufs=1) as fin, \
               tc.tile_pool(name="fin2", bufs=2) as fin2, \
               tc.tile_pool(name="fin_ps", bufs=2, space="PSUM") as fps:
              wo_t = fin.tile([64, 8 * E], dt.float32r, tag="wo_t")
              nc.scalar.dma_start(
                  wo_t[:].rearrange("p (c e) -> p c e", c=8),
                  wo_d.rearrange("(c p) e -> p c e", p=64))
              crow = fin.tile([1, E], dt.float32r, tag="crow")
              nc.scalar.dma_start(crow[:], crow_d[:, :])
              onesc = fin.tile([1, 128], dt.float32r, tag="onesc")
              nc.scalar.dma_start(onesc[:], ones_d[:, :])

              if upto == 'all':
                  with nc.allow_low_precision(reason="f32r view of f32 reciprocal"):
                      nc.vector.reciprocal(rcp[:], dall[:])
              for h in range(H if upto == 'all' else 0):
                  bc = fps.tile([64, SQ], dt.float32, tag="bc")
                  nc.tensor.matmul(bc[:], sel[:, h * 64:(h + 1) * 64], rcp[:],
                                   start=True, stop=True)
                  nc.vector.scalar_tensor_tensor(oT[h][:], oT[h][:], 1.0, bc[:],
                                                 Alu.mult, Alu.mult)
              for st in range(NQT if upto == 'all' else 1):
                  fo = fps.tile([128, E], dt.float32, tag="fo")
                  for ec8 in range(8):
                      nc.tensor.matmul(
                          fo[:], oT[ec8][:, st * 128:(st + 1) * 128],
                          wo_t[:, ec8 * E:(ec8 + 1) * E],
                          start=(ec8 == 0), stop=False)
                  nc.tensor.matmul(fo[:], onesc[:], crow[:],
                                   start=False, stop=True)
                  ot = fin2.tile([128, E], dt.float32, tag="ot")
                  nc.scalar.activation(ot[:], fo[:], Act.Identity)
                  nc.scalar.dma_start(
                      out_d.rearrange("(st p) e -> st p e", p=128)[st], ot[:])

    if waitfix:
        split_multi_waits(nc)
    return nc


_cache = {}


def kernel(query, key, value, mask, dict_mask, wq, bq, wk, bk, wv, bv, wo, bo,
           head_weights):
    query = np.asarray(query, np.float32)
    key = np.asarray(key, np.float32)
    value = np.asarray(value, np.float32)
    mask = np.asarray(mask, np.int32)
    dict_mask = np.asarray(dict_mask, np.float32)
    wq, bq = np.asarray(wq, np.float32), np.asarray(bq, np.float32)
    wk, bk = np.asarray(wk, np.float32), np.asarray(bk, np.float32)
    wv, bv = np.asarray(wv, np.float32), np.asarray(bv, np.float32)
    wo, bo = np.asarray(wo, np.float32), np.asarray(bo, np.float32)
    hw = np.asarray(head_weights, np.float32)

    # dm_h = scale_h * (d0 + r_h * d1) with scale_h = hw0, r_h = hw1/hw0
    consts = np.zeros((128, 16), np.float32)
    for h in range(H):
        a, b_ = float(hw[h, 0]), float(hw[h, 1])
        if abs(a) < 1e-20:
            a = 1e-20 if a >= 0 else -1e-20
        consts[:, h] = b_ / a
        consts[:, 8 + h] = a

    bqs = np.ascontiguousarray(0.125 * bq.reshape(NEC, 128).T)
    bks = np.ascontiguousarray(bk.reshape(NEC, 128).T)
    crow = (bv @ wo + bo).reshape(1, E).astype(np.float32)
    sel_np = np.zeros((8, H * 64), np.float32)
    eye_np = np.zeros((1, H * 8), np.float32)
    for h in range(H):
        sel_np[h, h * 64:(h + 1) * 64] = 1.0
        eye_np[0, h * 8 + h] = 1.0

    if "nc" not in _cache:
        _cache["nc"] = build()
    nc = _cache["nc"]

    in_maps = _in_maps(query, key, value, mask, dict_mask, consts, bqs, bks,
                       crow, sel_np, eye_np, wq, wk, wv, wo)

    res = run_bass_kernel_spmd(nc, in_maps, core_ids=list(range(NCORE)))
    out = np.empty((B, S, E), np.float32)
    for c in range(NCORE):
        b = c // 4
        qs = (c % 4) * SQ
        out[b, qs:qs + SQ] = res.results[c]["out_d"]
    return out


def _in_maps(query, key, value, mask, dict_mask, consts, bqs, bks, crow,
             sel_np, eye_np, wq, wk, wv, wo):
    in_maps = []
    for c in range(NCORE):
        b = c // 4
        qs = (c % 4) * SQ
        in_maps.append({
            "query_s": np.ascontiguousarray(query[b, qs:qs + SQ]),
            "key_b": np.ascontiguousarray(key[b]),
            "value_b": np.ascontiguousarray(value[b]),
            "mask_s": np.ascontiguousarray(mask[b, qs:qs + SQ]),
            "d0_s": np.ascontiguousarray(dict_mask[0, b, qs:qs + SQ]),
            "d1_s": np.ascontiguousarray(dict_mask[1, b, qs:qs + SQ]),
            "wq_d": wq, "wk_d": wk, "wv_d": wv, "wo_d": wo,
            "consts_d": consts, "bqs_d": bqs, "bks_d": bks, "crow_d": crow,
            "sel_d": sel_np, "eye_d": eye_np,
            "ones_d": np.ones((1, 128), np.float32),
        })
    return in_maps


def make_in_maps(inputs):
    """Rebuild the per-core input maps from the full input dict (test helper)."""
    hw = np.asarray(inputs["head_weights"], np.float32)
    consts = np.zeros((128, 16), np.float32)
    for h in range(H):
        a, b_ = float(hw[h, 0]), float(hw[h, 1])
        if abs(a) < 1e-20:
            a = 1e-20 if a >= 0 else -1e-20
        consts[:, h] = b_ / a
        consts[:, 8 + h] = a
    bq = np.asarray(inputs["bq"], np.float32)
    bk = np.asarray(inputs["bk"], np.float32)
    bv = np.asarray(inputs["bv"], np.float32)
    bo = np.asarray(inputs["bo"], np.float32)
    wo = np.asarray(inputs["wo"], np.float32)
    bqs = np.ascontiguousarray(0.125 * bq.reshape(NEC, 128).T)
    bks = np.ascontiguousarray(bk.reshape(NEC, 128).T)
    crow = (bv @ wo + bo).reshape(1, E).astype(np.float32)
    sel_np = np.zeros((8, H * 64), np.float32)
    eye_np = np.zeros((1, H * 8), np.float32)
    for h in range(H):
        sel_np[h, h * 64:(h + 1) * 64] = 1.0
        eye_np[0, h * 8 + h] = 1.0
    if "nc" not in _cache:
        _cache["nc"] = build()
    return _in_maps(np.asarray(inputs["query"], np.float32),
                    np.asarray(inputs["key"], np.float32),
                    np.asarray(inputs["value"], np.float32),
                    np.asarray(inputs["mask"], np.int32),
                    np.asarray(inputs["dict_mask"], np.float32),
                    consts, bqs, bks, crow, sel_np, eye_np,
                    np.asarray(inputs["wq"], np.float32),
                    np.asarray(inputs["wk"], np.float32),
                    np.asarray(inputs["wv"], np.float32), wo)

